# revision 2
# baseline (speedup 1.0000x reference)
"""Trainium2 Bass kernel for nn_AttentionGCNLayer (B=2, N=4096, D=256, H=2, ITERS=2).

Sharding: 8 cores = (b in 2) x (h in 2) x (row-half in 2). Each core handles one
(batch, head) pair and one half (2048) of the attention rows, with a pairwise
AllGather of the updated node features between the two GCN iterations.

Layout choices (per core):
  - x is kept transposed (x^T, [D=2x128 partitions, N free]) in *local* row
    order: columns [0:2048) are this core's rows, [2048:4096) the partner's.
    The aggregation sum over neighbors is permutation invariant, so local
    ordering is consistent as long as k/h/E all use the same order (they do).
  - scores are computed transposed (E^T = exp(q k^T / sqrt(dk))^T with
    [neighbor n on partitions, attention rows on free]) so that the
    neighbor-aggregation matmul consumes E^T directly, with no transposes.
  - softmax normalizer: rows of exp(scores) are summed with a ones-vector
    matmul on the PE; attn @ h / degs == (E @ h) * (1/R) with R = rowsum(E)
    (degs == 1 up to fp rounding, matching the reference within fp32 noise).
    scores are in [-1, 1], so exp needs no max-subtraction.
  - big matmuls run in bf16 (inputs) with fp32 PSUM accumulation.
"""

import sys

if "/opt/trn_rl_repo" not in sys.path:
    sys.path.insert(0, "/opt/trn_rl_repo")

import numpy as np

B, N, D, H, ITERS = 2, 4096, 256, 2, 2
DK = D // H                      # 128
RH = N // 2                      # 2048 rows per core
NCH = N // 128                   # 32 neighbor chunks
HCH = NCH // 2                   # 16 chunks per half
RT = 512                         # row tile (one PSUM bank of fp32)
NRT = RH // RT                   # 4 row tiles per core
SCALE = 1.0 / float(np.sqrt(np.float32(DK)))

_CACHE = {}


def _seq_engines(mybir):
    return {
        mybir.EngineType.PE,
        mybir.EngineType.Activation,
        mybir.EngineType.Pool,
        mybir.EngineType.DVE,
        mybir.EngineType.SP,
    }


def _split_excess_waits(nc, mybir, max_waits=1):
    """This container's walrus accepts at most one sync-wait per engine
    instruction; hoist extra waits onto preceding NoOps on the same engine."""
    seq = _seq_engines(mybir)
    n_new = 0
    for f in nc.m.functions:
        for blk in f.blocks:
            if not any(
                inst.sync_info is not None
                and inst.sync_info.on_wait
                and len(inst.sync_info.on_wait) > max_waits
                and inst.engine in seq
                for inst in blk.instructions
            ):
                continue
            out = []
            for inst in blk.instructions:
                si = inst.sync_info
                if (
                    si is not None
                    and si.on_wait
                    and len(si.on_wait) > max_waits
                    and inst.engine in seq
                ):
                    waits = list(si.on_wait)
                    keep, extra = waits[:max_waits], waits[max_waits:]
                    while extra:
                        chunk, extra = extra[:max_waits], extra[max_waits:]
                        out.append(
                            mybir.InstNoOp(
                                name=f"{inst.name}-ws{n_new}",
                                sync_info=mybir.SyncInfo(on_wait=chunk, on_update=[]),
                                bass_nofuse=True,
                                engine=inst.engine,
                            )
                        )
                        n_new += 1
                    inst.sync_info = mybir.SyncInfo(
                        on_wait=keep, on_update=list(si.on_update)
                    )
                out.append(inst)
            blk.instructions = out
    return n_new


def _build():
    import concourse.bass as bass
    import concourse.mybir as mybir
    import concourse.tile as tile
    from concourse.masks import make_identity

    f32 = mybir.dt.float32
    bf16 = mybir.dt.bfloat16
    AF = mybir.ActivationFunctionType

    nc = bass.Bass("TRN2", num_devices=8)

    nodes = nc.dram_tensor("nodes", [N, D], f32, kind="ExternalInput")
    wq = nc.dram_tensor("wq", [D, DK], f32, kind="ExternalInput")
    wk = nc.dram_tensor("wk", [D, DK], f32, kind="ExternalInput")
    wqb = nc.dram_tensor("wqb", [DK, 1], f32, kind="ExternalInput")
    wkb = nc.dram_tensor("wkb", [DK, 1], f32, kind="ExternalInput")
    gw = nc.dram_tensor("gw", [ITERS, D, D], f32, kind="ExternalInput")
    gb = nc.dram_tensor("gb", [ITERS, 2, 128, 1], f32, kind="ExternalInput")
    agg = nc.dram_tensor("agg", [D, D], f32, kind="ExternalInput")
    m0d = nc.dram_tensor("m0", [128, 1], f32, kind="ExternalInput")
    m1d = nc.dram_tensor("m1", [128, 1], f32, kind="ExternalInput")
    part = nc.dram_tensor("part", [RH, D], f32, kind="ExternalOutput")

    with tile.TileContext(nc) as tc:
        from contextlib import ExitStack

        with ExitStack() as ctx:
            const = ctx.enter_context(tc.tile_pool(name="const", bufs=1))

            ident = const.tile([128, 128], f32, name="ident")
            make_identity(nc, ident)
            ones_col = const.tile([128, 1], bf16, name="ones_col")
            nc.vector.memset(ones_col, 1.0)
            ones_row = const.tile([1, 128], f32, name="ones_row")
            nc.vector.memset(ones_row, 1.0)

            # persistent state
            # x^T in bf16, split by feature chunk (dc) and row half (a=mine, b=partner)
            xT = [
                [
                    const.tile([128, RH], bf16, name=f"xT{dc}{hf}")
                    for hf in range(2)
                ]
                for dc in range(2)
            ]
            eT = [const.tile([128, RH], bf16, name=f"eT{i}") for i in range(NCH)]
            rinvB = const.tile([128, RH], f32, name="rinvB")

            # small weights/biases
            wq_s = const.tile([128, 2, DK], bf16, name="wq_s")
            wk_s = const.tile([128, 2, DK], bf16, name="wk_s")
            gw_s = const.tile([128, ITERS, 2, D], bf16, name="gw_s")
            agg_s = const.tile([128, 2, D], bf16, name="agg_s")
            wqb_s = const.tile([128, 1], f32, name="wqb_s")
            wkb_s = const.tile([128, 1], f32, name="wkb_s")
            gb_s = const.tile([128, ITERS, 2, 1], f32, name="gb_s")
            m0_s = const.tile([128, 1], f32, name="m0_s")
            m1_s = const.tile([128, 1], f32, name="m1_s")

            nc.sync.dma_start(out=wqb_s, in_=wqb[:, :])
            nc.sync.dma_start(out=wkb_s, in_=wkb[:, :])
            nc.sync.dma_start(out=m0_s, in_=m0d[:, :])
            nc.sync.dma_start(out=m1_s, in_=m1d[:, :])
            for i in range(ITERS):
                for dc in range(2):
                    nc.sync.dma_start(out=gb_s[:, i, dc, :], in_=gb[i, dc, :, :])

            # ---- P0: stage + cast weights, transpose nodes into x^T ----
            with tc.tile_pool(name="stg", bufs=4) as stg, tc.tile_pool(
                name="ps_tr", bufs=4, space="PSUM"
            ) as ps_tr:
                for dc in range(2):
                    ws = stg.tile([128, DK], f32, name="wstg", tag="wstg")
                    nc.sync.dma_start(out=ws, in_=wq[dc * 128 : (dc + 1) * 128, :])
                    nc.vector.tensor_copy(out=wq_s[:, dc, :], in_=ws)
                    ws2 = stg.tile([128, DK], f32, name="wstg2", tag="wstg")
                    nc.sync.dma_start(out=ws2, in_=wk[dc * 128 : (dc + 1) * 128, :])
                    nc.vector.tensor_copy(out=wk_s[:, dc, :], in_=ws2)
                for i in range(ITERS):
                    for dc in range(2):
                        ws = stg.tile([128, D], f32, name="gstg", tag="gstg")
                        nc.sync.dma_start(
                            out=ws, in_=gw[i, dc * 128 : (dc + 1) * 128, :]
                        )
                        nc.vector.tensor_copy(out=gw_s[:, i, dc, :], in_=ws)
                for dc in range(2):
                    ws = stg.tile([128, D], f32, name="astg", tag="gstg")
                    nc.sync.dma_start(out=ws, in_=agg[dc * 128 : (dc + 1) * 128, :])
                    nc.vector.tensor_copy(out=agg_s[:, dc, :], in_=ws)

                # nodes -> x^T (bf16) via PE transpose
                for ch in range(NCH):
                    hf, col = (0, ch * 128) if ch < HCH else (1, (ch - HCH) * 128)
                    st = stg.tile([128, D], f32, name="nstg", tag="nstg")
                    nc.sync.dma_start(
                        out=st, in_=nodes[ch * 128 : (ch + 1) * 128, :]
                    )
                    for dc in range(2):
                        pt = ps_tr.tile([128, 128], f32, name="ptr", tag="ptr")
                        nc.tensor.transpose(
                            pt, st[:, dc * 128 : (dc + 1) * 128], ident
                        )
                        nc.vector.tensor_copy(
                            out=xT[dc][hf][:, col : col + 128], in_=pt
                        )

            # ---- P1: k^T, q^T, E^T = exp(scores^T), R ----
            with tc.tile_pool(name="kq", bufs=1) as kq:
                kT = kq.tile([128, N], bf16, name="kT")
                qT = kq.tile([128, RH], bf16, name="qT")

                with tc.tile_pool(name="ps_qk", bufs=3, space="PSUM") as ps_qk:
                    for ct in range(N // RT):  # k for all rows (local order)
                        hf, col = (0, ct * RT) if ct < NRT else (1, (ct - NRT) * RT)
                        ps = ps_qk.tile([128, RT], f32, name="psk", tag="psqk")
                        for dc in range(2):
                            nc.tensor.matmul(
                                ps,
                                wk_s[:, dc, :],
                                xT[dc][hf][:, col : col + RT],
                                start=(dc == 0),
                                stop=(dc == 1),
                            )
                        nc.scalar.activation(
                            out=kT[:, ct * RT : (ct + 1) * RT],
                            in_=ps,
                            func=AF.Identity,
                            bias=wkb_s,
                            scale=1.0,
                        )
                    for ct in range(NRT):  # q for my rows only
                        ps = ps_qk.tile([128, RT], f32, name="psq", tag="psqk")
                        for dc in range(2):
                            nc.tensor.matmul(
                                ps,
                                wq_s[:, dc, :],
                                xT[dc][0][:, ct * RT : (ct + 1) * RT],
                                start=(dc == 0),
                                stop=(dc == 1),
                            )
                        nc.scalar.activation(
                            out=qT[:, ct * RT : (ct + 1) * RT],
                            in_=ps,
                            func=AF.Identity,
                            bias=wqb_s,
                            scale=1.0,
                        )

                with tc.tile_pool(
                    name="ps_sc", bufs=4, space="PSUM"
                ) as ps_sc, tc.tile_pool(
                    name="ps_r", bufs=2, space="PSUM"
                ) as ps_r, tc.tile_pool(
                    name="ps_rb", bufs=2, space="PSUM"
                ) as ps_rb, tc.tile_pool(name="rr", bufs=2) as rr:
                    for rt in range(NRT):
                        rsum = ps_r.tile([1, RT], f32, name="rsum", tag="rsum")
                        for ncx in range(NCH):
                            ps = ps_sc.tile([128, RT], f32, name="pss", tag="pss")
                            nc.tensor.matmul(
                                ps,
                                kT[:, ncx * 128 : (ncx + 1) * 128],
                                qT[:, rt * RT : (rt + 1) * RT],
                                start=True,
                                stop=True,
                            )
                            nc.scalar.activation(
                                out=eT[ncx][:, rt * RT : (rt + 1) * RT],
                                in_=ps,
                                func=AF.Exp,
                                scale=SCALE,
                            )
                            nc.tensor.matmul(
                                rsum,
                                ones_col,
                                eT[ncx][:, rt * RT : (rt + 1) * RT],
                                start=(ncx == 0),
                                stop=(ncx == NCH - 1),
                            )
                        rrow = rr.tile([1, RT], f32, name="rrow", tag="rrow")
                        nc.vector.reciprocal(out=rrow, in_=rsum)
                        psb = ps_rb.tile([128, RT], f32, name="psb", tag="psb")
                        nc.tensor.matmul(psb, ones_row, rrow, start=True, stop=True)
                        nc.vector.tensor_copy(
                            out=rinvB[:, rt * RT : (rt + 1) * RT], in_=psb
                        )

            # ---- P2/P3: GCN iterations ----
            with tc.tile_pool(name="hp", bufs=1) as hp, tc.tile_pool(
                name="dram", bufs=1, space="DRAM"
            ) as dram:
                hS = [hp.tile([128, D], bf16, name=f"hS{i}") for i in range(NCH)]
                cc_in = dram.tile([2 * 128, RH], bf16, name="cc_in")
                cc_out = dram.tile([4 * 128, RH], bf16, name="cc_out")

                for it in range(ITERS):
                    # h = x @ gcn_W[it]   (chunks over neighbors, local order)
                    with tc.tile_pool(
                        name=f"ps_h{it}", bufs=3, space="PSUM"
                    ) as ps_h:
                        for ncx in range(NCH):
                            hf, col = (
                                (0, ncx * 128)
                                if ncx < HCH
                                else (1, (ncx - HCH) * 128)
                            )
                            ps = ps_h.tile([128, D], f32, name="psh", tag="psh")
                            for dc in range(2):
                                nc.tensor.matmul(
                                    ps,
                                    xT[dc][hf][:, col : col + 128],
                                    gw_s[:, it, dc, :],
                                    start=(dc == 0),
                                    stop=(dc == 1),
                                )
                            nc.scalar.activation(
                                out=hS[ncx], in_=ps, func=AF.Copy
                            )

                    # U^T = sum_n h^T E^T ; x_mine += relu(U/R + b)
                    with tc.tile_pool(
                        name=f"ps_u{it}", bufs=4, space="PSUM"
                    ) as ps_u, tc.tile_pool(name=f"upd{it}", bufs=3) as upd:
                        for rt in range(NRT):
                            pu = [
                                ps_u.tile([128, RT], f32, name=f"pu{dc}", tag="pu")
                                for dc in range(2)
                            ]
                            for ncx in range(NCH):
                                for dc in range(2):
                                    nc.tensor.matmul(
                                        pu[dc],
                                        hS[ncx][:, dc * 128 : (dc + 1) * 128],
                                        eT[ncx][:, rt * RT : (rt + 1) * RT],
                                        start=(ncx == 0),
                                        stop=(ncx == NCH - 1),
                                    )
                            for dc in range(2):
                                t = upd.tile([128, RT], f32, name="updt", tag="updt")
                                nc.vector.tensor_mul(
                                    t, pu[dc], rinvB[:, rt * RT : (rt + 1) * RT]
                                )
                                nc.scalar.activation(
                                    out=t,
                                    in_=t,
                                    func=AF.Relu,
                                    bias=gb_s[:, it, dc, :],
                                    scale=1.0,
                                )
                                nc.vector.tensor_add(
                                    out=xT[dc][0][:, rt * RT : (rt + 1) * RT],
                                    in0=xT[dc][0][:, rt * RT : (rt + 1) * RT],
                                    in1=t,
                                )

                    if it == 0:
                        # exchange updated halves with the partner core
                        for dc in range(2):
                            nc.sync.dma_start(
                                out=cc_in[dc * 128 : (dc + 1) * 128, :],
                                in_=xT[dc][0][:, :],
                            )
                        nc.gpsimd.collective_compute(
                            "AllGather",
                            mybir.AluOpType.bypass,
                            replica_groups=[[0, 1], [2, 3], [4, 5], [6, 7]],
                            ins=[cc_in[:, :].opt()],
                            outs=[cc_out[:, :].opt()],
                        )
                        # partner half = rank0_block * m1 + rank1_block * m0
                        with tc.tile_pool(name="cct", bufs=4) as cct:
                            for ct in range(NRT):
                                for dc in range(2):
                                    t0 = cct.tile(
                                        [128, RT], bf16, name="t0", tag="cct"
                                    )
                                    t1 = cct.tile(
                                        [128, RT], bf16, name="t1", tag="cct"
                                    )
                                    nc.sync.dma_start(
                                        out=t0,
                                        in_=cc_out[
                                            dc * 128 : (dc + 1) * 128,
                                            ct * RT : (ct + 1) * RT,
                                        ],
                                    )
                                    nc.sync.dma_start(
                                        out=t1,
                                        in_=cc_out[
                                            256 + dc * 128 : 256 + (dc + 1) * 128,
                                            ct * RT : (ct + 1) * RT,
                                        ],
                                    )
                                    nc.vector.tensor_scalar_mul(t0, t0, m1_s)
                                    nc.vector.tensor_scalar_mul(t1, t1, m0_s)
                                    nc.vector.tensor_add(
                                        out=xT[dc][1][:, ct * RT : (ct + 1) * RT],
                                        in0=t0,
                                        in1=t1,
                                    )

                # ---- output partial: part = x2[mine] @ aggW_h ----
                with tc.tile_pool(
                    name="ps_o", bufs=3, space="PSUM"
                ) as ps_o, tc.tile_pool(name="ost", bufs=3) as ost:
                    for rc in range(RH // 128):
                        ps = ps_o.tile([128, D], f32, name="pso", tag="pso")
                        for dc in range(2):
                            nc.tensor.matmul(
                                ps,
                                xT[dc][0][:, rc * 128 : (rc + 1) * 128],
                                agg_s[:, dc, :],
                                start=(dc == 0),
                                stop=(dc == 1),
                            )
                        ot = ost.tile([128, D], f32, name="ot", tag="ot")
                        nc.scalar.activation(out=ot, in_=ps, func=AF.Copy)
                        nc.sync.dma_start(
                            out=part[rc * 128 : (rc + 1) * 128, :], in_=ot
                        )

    _split_excess_waits(nc, mybir)
    return nc


def _get_nc():
    if "nc" not in _CACHE:
        _CACHE["nc"] = _build()
    return _CACHE["nc"]


def _in_maps(inputs):
    ne = np.asarray(inputs["nodes_embed"], dtype=np.float32)
    wq_w = np.asarray(inputs["WQ_w"], dtype=np.float32)
    wq_b = np.asarray(inputs["WQ_b"], dtype=np.float32)
    wk_w = np.asarray(inputs["WK_w"], dtype=np.float32)
    wk_b = np.asarray(inputs["WK_b"], dtype=np.float32)
    gcn_w = np.asarray(inputs["gcn_W"], dtype=np.float32)
    gcn_b = np.asarray(inputs["gcn_b"], dtype=np.float32)
    agg_w = np.asarray(inputs["agg_W"], dtype=np.float32)

    gb = np.ascontiguousarray(gcn_b.reshape(ITERS, 2, 128, 1))
    maps = []
    for c in range(8):
        b, h, rh = c // 4, (c // 2) % 2, c % 2
        if rh == 0:
            nodes = ne[b]
        else:
            nodes = np.concatenate([ne[b, RH:], ne[b, :RH]], axis=0)
        m0 = np.full((128, 1), 1.0 if rh == 0 else 0.0, np.float32)
        m1 = np.full((128, 1), 0.0 if rh == 0 else 1.0, np.float32)
        maps.append(
            {
                "nodes": np.ascontiguousarray(nodes),
                "wq": np.ascontiguousarray(wq_w[:, h * DK : (h + 1) * DK]),
                "wk": np.ascontiguousarray(wk_w[:, h * DK : (h + 1) * DK]),
                "wqb": np.ascontiguousarray(
                    wq_b[h * DK : (h + 1) * DK].reshape(DK, 1)
                ),
                "wkb": np.ascontiguousarray(
                    wk_b[h * DK : (h + 1) * DK].reshape(DK, 1)
                ),
                "gw": gcn_w,
                "gb": gb,
                "agg": np.ascontiguousarray(agg_w[h * D : (h + 1) * D, :]),
                "m0": m0,
                "m1": m1,
            }
        )
    return maps


def kernel(trace=False, tmpdir=None, **inputs):
    from concourse.bass_utils import run_bass_kernel_spmd

    nc = _get_nc()
    maps = _in_maps(inputs)
    kw = {}
    if trace:
        kw = dict(trace=True, tmpdir=tmpdir)
    res = run_bass_kernel_spmd(nc, maps, core_ids=list(range(8)), **kw)

    agg_b = np.asarray(inputs["agg_b"], dtype=np.float32)
    out = np.zeros((B, N, D), np.float32)
    for b in range(B):
        for rh in range(2):
            rows = slice(rh * RH, (rh + 1) * RH)
            out[b, rows, :] = (
                res.results[4 * b + 0 * 2 + rh]["part"]
                + res.results[4 * b + 1 * 2 + rh]["part"]
                + agg_b
            )
    if trace:
        return out, res
    return out


# revision 6
# speedup vs baseline: 1.2304x; 1.2304x over previous
"""Trainium2 Bass kernel for nn_AttentionGCNLayer (B=2, N=4096, D=256, H=2, ITERS=2).

Sharding: 8 cores = (b in 2) x (h in 2) x (row-half in 2). Each core handles one
(batch, head) pair and one half (2048) of the attention rows, with a pairwise
AllGather of the updated node features between the two GCN iterations.

Layout choices (per core):
  - x is kept transposed (x^T, [D=2x128 partitions, N free]) in *local* row
    order: columns [0:2048) are this core's rows, [2048:4096) the partner's.
    The aggregation sum over neighbors is permutation invariant, so local
    ordering is consistent as long as k/h/E all use the same order (they do).
  - scores are computed transposed (E^T = exp(q k^T / sqrt(dk))^T with
    [neighbor n on partitions, attention rows on free]) so that the
    neighbor-aggregation matmul consumes E^T directly, with no transposes.
  - softmax normalizer: rows of exp(scores) are summed with a ones-vector
    matmul on the PE; attn @ h / degs == (E @ h) * (1/R) with R = rowsum(E)
    (degs == 1 up to fp rounding, matching the reference within fp32 noise).
    scores are in [-1, 1], so exp needs no max-subtraction.
  - big matmuls run in bf16 (inputs) with fp32 PSUM accumulation.
"""

import sys

if "/opt/trn_rl_repo" not in sys.path:
    sys.path.insert(0, "/opt/trn_rl_repo")

import numpy as np

B, N, D, H, ITERS = 2, 4096, 256, 2, 2
DK = D // H                      # 128
RH = N // 2                      # 2048 rows per core
NCH = N // 128                   # 32 neighbor chunks
HCH = NCH // 2                   # 16 chunks per half
RT = 512                         # row tile (one PSUM bank of fp32)
NRT = RH // RT                   # 4 row tiles per core
SCALE = 1.0 / float(np.sqrt(np.float32(DK)))

_CACHE = {}


def _seq_engines(mybir):
    return {
        mybir.EngineType.PE,
        mybir.EngineType.Activation,
        mybir.EngineType.Pool,
        mybir.EngineType.DVE,
        mybir.EngineType.SP,
    }


def _split_excess_waits(nc, mybir, max_waits=1):
    """This container's walrus accepts at most one sync-wait per engine
    instruction; hoist extra waits onto preceding NoOps on the same engine."""
    seq = _seq_engines(mybir)
    n_new = 0
    for f in nc.m.functions:
        for blk in f.blocks:
            if not any(
                inst.sync_info is not None
                and inst.sync_info.on_wait
                and len(inst.sync_info.on_wait) > max_waits
                and inst.engine in seq
                for inst in blk.instructions
            ):
                continue
            out = []
            for inst in blk.instructions:
                si = inst.sync_info
                if (
                    si is not None
                    and si.on_wait
                    and len(si.on_wait) > max_waits
                    and inst.engine in seq
                ):
                    waits = list(si.on_wait)
                    keep, extra = waits[:max_waits], waits[max_waits:]
                    while extra:
                        chunk, extra = extra[:max_waits], extra[max_waits:]
                        out.append(
                            mybir.InstNoOp(
                                name=f"{inst.name}-ws{n_new}",
                                sync_info=mybir.SyncInfo(on_wait=chunk, on_update=[]),
                                bass_nofuse=True,
                                engine=inst.engine,
                            )
                        )
                        n_new += 1
                    inst.sync_info = mybir.SyncInfo(
                        on_wait=keep, on_update=list(si.on_update)
                    )
                out.append(inst)
            blk.instructions = out
    return n_new


def _build():
    import concourse.bass as bass
    import concourse.mybir as mybir
    import concourse.tile as tile
    from concourse.masks import make_identity

    f32 = mybir.dt.float32
    bf16 = mybir.dt.bfloat16
    AF = mybir.ActivationFunctionType

    nc = bass.Bass("TRN2", num_devices=8)

    nodes = nc.dram_tensor("nodes", [N, D], f32, kind="ExternalInput")
    wq = nc.dram_tensor("wq", [D, DK], f32, kind="ExternalInput")
    wk = nc.dram_tensor("wk", [D, DK], f32, kind="ExternalInput")
    wqb = nc.dram_tensor("wqb", [DK, 1], f32, kind="ExternalInput")
    wkb = nc.dram_tensor("wkb", [DK, 1], f32, kind="ExternalInput")
    gw = nc.dram_tensor("gw", [ITERS, D, D], f32, kind="ExternalInput")
    gb = nc.dram_tensor("gb", [ITERS, 2, 128, 1], f32, kind="ExternalInput")
    agg = nc.dram_tensor("agg", [D, D], f32, kind="ExternalInput")
    m0d = nc.dram_tensor("m0", [128, 1], f32, kind="ExternalInput")
    m1d = nc.dram_tensor("m1", [128, 1], f32, kind="ExternalInput")
    part = nc.dram_tensor("part", [RH, D], f32, kind="ExternalOutput")

    with tile.TileContext(nc) as tc:
        from contextlib import ExitStack

        with ExitStack() as ctx:
            const = ctx.enter_context(tc.tile_pool(name="const", bufs=1))

            ident = const.tile([128, 128], f32, name="ident")
            make_identity(nc, ident)
            ones_col = const.tile([128, 1], f32, name="ones_col")
            nc.vector.memset(ones_col, 1.0)
            ones_row = const.tile([1, 128], f32, name="ones_row")
            nc.vector.memset(ones_row, 1.0)

            # persistent state
            # x^T in bf16, split by feature chunk (dc) and row half (a=mine, b=partner)
            xT = [
                [
                    const.tile([128, RH], bf16, name=f"xT{dc}{hf}")
                    for hf in range(2)
                ]
                for dc in range(2)
            ]
            eT = [const.tile([128, RH], bf16, name=f"eT{i}") for i in range(NCH)]
            rinvB = const.tile([128, RH], f32, name="rinvB")

            # small weights/biases
            wq_s = const.tile([128, 2, DK], bf16, name="wq_s")
            wk_s = const.tile([128, 2, DK], bf16, name="wk_s")
            gw_s = const.tile([128, ITERS, 2, D], bf16, name="gw_s")
            agg_s = const.tile([128, 2, D], bf16, name="agg_s")
            wqb_s = const.tile([128, 1], f32, name="wqb_s")
            wkb_s = const.tile([128, 1], f32, name="wkb_s")
            gb_s = const.tile([128, ITERS, 2, 1], f32, name="gb_s")
            m0_s = const.tile([128, 1], f32, name="m0_s")
            m1_s = const.tile([128, 1], f32, name="m1_s")

            nc.sync.dma_start(out=wqb_s, in_=wqb[:, :])
            nc.sync.dma_start(out=wkb_s, in_=wkb[:, :])
            nc.sync.dma_start(out=m0_s, in_=m0d[:, :])
            nc.sync.dma_start(out=m1_s, in_=m1d[:, :])
            for i in range(ITERS):
                for dc in range(2):
                    nc.sync.dma_start(out=gb_s[:, i, dc, :], in_=gb[i, dc, :, :])

            # ---- P0: stage + cast weights, transpose nodes into x^T ----
            with tc.tile_pool(name="stg", bufs=4) as stg, tc.tile_pool(
                name="ps_tr", bufs=4, space="PSUM"
            ) as ps_tr:
                for dc in range(2):
                    ws = stg.tile([128, DK], f32, name="wstg", tag="wstg")
                    nc.sync.dma_start(out=ws, in_=wq[dc * 128 : (dc + 1) * 128, :])
                    nc.vector.tensor_copy(out=wq_s[:, dc, :], in_=ws)
                    ws2 = stg.tile([128, DK], f32, name="wstg2", tag="wstg")
                    nc.sync.dma_start(out=ws2, in_=wk[dc * 128 : (dc + 1) * 128, :])
                    nc.vector.tensor_copy(out=wk_s[:, dc, :], in_=ws2)
                for i in range(ITERS):
                    for dc in range(2):
                        ws = stg.tile([128, D], f32, name="gstg", tag="gstg")
                        nc.sync.dma_start(
                            out=ws, in_=gw[i, dc * 128 : (dc + 1) * 128, :]
                        )
                        nc.vector.tensor_copy(out=gw_s[:, i, dc, :], in_=ws)
                for dc in range(2):
                    ws = stg.tile([128, D], f32, name="astg", tag="gstg")
                    nc.sync.dma_start(out=ws, in_=agg[dc * 128 : (dc + 1) * 128, :])
                    nc.vector.tensor_copy(out=agg_s[:, dc, :], in_=ws)

                # nodes -> x^T (bf16) via PE transpose
                for ch in range(NCH):
                    hf, col = (0, ch * 128) if ch < HCH else (1, (ch - HCH) * 128)
                    st = stg.tile([128, D], f32, name="nstg", tag="nstg")
                    nc.sync.dma_start(
                        out=st, in_=nodes[ch * 128 : (ch + 1) * 128, :]
                    )
                    for dc in range(2):
                        pt = ps_tr.tile([128, 128], f32, name="ptr", tag="ptr")
                        nc.tensor.transpose(
                            pt, st[:, dc * 128 : (dc + 1) * 128], ident
                        )
                        nc.vector.tensor_copy(
                            out=xT[dc][hf][:, col : col + 128], in_=pt
                        )

            # ---- P1 + GCN, rowtile-pipelined ----
            kq = ctx.enter_context(tc.tile_pool(name="kq", bufs=1))
            kT = kq.tile([128, N], bf16, name="kT")
            qT = kq.tile([128, RH], bf16, name="qT")

            ps_sc = ctx.enter_context(tc.tile_pool(name="ps_sc", bufs=3, space="PSUM"))
            ps_h = ctx.enter_context(tc.tile_pool(name="ps_h", bufs=2, space="PSUM"))
            ps_u = ctx.enter_context(tc.tile_pool(name="ps_u", bufs=2, space="PSUM"))

            if True:
                for ct in range(N // RT):  # k for all rows (local order)
                    hf, col = (0, ct * RT) if ct < NRT else (1, (ct - NRT) * RT)
                    ps = ps_sc.tile([128, RT], f32, name="psk", tag="pss")
                    for dc in range(2):
                        nc.tensor.matmul(
                            ps,
                            wk_s[:, dc, :],
                            xT[dc][hf][:, col : col + RT],
                            start=(dc == 0),
                            stop=(dc == 1),
                        )
                    nc.scalar.activation(
                        out=kT[:, ct * RT : (ct + 1) * RT],
                        in_=ps,
                        func=AF.Identity,
                        bias=wkb_s,
                        scale=1.0,
                    )
                for ct in range(NRT):  # q for my rows only
                    ps = ps_sc.tile([128, RT], f32, name="psq", tag="pss")
                    for dc in range(2):
                        nc.tensor.matmul(
                            ps,
                            wq_s[:, dc, :],
                            xT[dc][0][:, ct * RT : (ct + 1) * RT],
                            start=(dc == 0),
                            stop=(dc == 1),
                        )
                    nc.scalar.activation(
                        out=qT[:, ct * RT : (ct + 1) * RT],
                        in_=ps,
                        func=AF.Identity,
                        bias=wqb_s,
                        scale=1.0,
                    )

            hS = [const.tile([128, D], bf16, name=f"hS{i}") for i in range(NCH)]
            racc = ctx.enter_context(tc.tile_pool(name="racc", bufs=2))
            upd = ctx.enter_context(tc.tile_pool(name="upd", bufs=3))
            dram = ctx.enter_context(tc.tile_pool(name="dram", bufs=1, space="DRAM"))
            cc_in = dram.tile([2 * 128, RH], bf16, name="cc_in")
            cc_out = dram.tile([4 * 128, RH], bf16, name="cc_out")

            def scores_exp(rt):
                for ncx in range(NCH):
                    ps = ps_sc.tile([128, RT], f32, name="pss", tag="pss")
                    nc.tensor.matmul(
                        ps,
                        kT[:, ncx * 128 : (ncx + 1) * 128],
                        qT[:, rt * RT : (rt + 1) * RT],
                        start=True,
                        stop=True,
                    )
                    nc.scalar.activation(
                        out=eT[ncx][:, rt * RT : (rt + 1) * RT],
                        in_=ps,
                        func=AF.Exp,
                        scale=SCALE,
                    )

            def r_reduce(rt):
                # R = sum_n E (tree of DVE adds, f32), then 1-matmul partition
                # reduce, broadcast, reciprocal
                t = racc.tile([128, RT], f32, name="rp", tag="rp")
                nc.vector.tensor_add(
                    out=t,
                    in0=eT[0][:, rt * RT : (rt + 1) * RT],
                    in1=eT[1][:, rt * RT : (rt + 1) * RT],
                )
                for i in range(2, NCH):
                    nc.vector.tensor_add(
                        out=t, in0=t, in1=eT[i][:, rt * RT : (rt + 1) * RT]
                    )
                ps_row = ps_sc.tile([1, RT], f32, name="psrow", tag="psrow", bufs=1)
                nc.tensor.matmul(ps_row, ones_col, t, start=True, stop=True)
                rrow = racc.tile([1, RT], f32, name="rrow", tag="rrow")
                nc.vector.tensor_copy(out=rrow, in_=ps_row)
                ps_b = ps_sc.tile([128, RT], f32, name="psb", tag="pss")
                nc.tensor.matmul(ps_b, ones_row, rrow, start=True, stop=True)
                nc.vector.reciprocal(
                    out=rinvB[:, rt * RT : (rt + 1) * RT], in_=ps_b
                )

            def h_gen(it, half):
                rng = range(HCH) if half == 0 else range(HCH, NCH)
                for ncx in rng:
                    hf, col = (
                        (0, ncx * 128) if ncx < HCH else (1, (ncx - HCH) * 128)
                    )
                    ps = ps_h.tile([128, D], f32, name="psh", tag="psh")
                    for dc in range(2):
                        nc.tensor.matmul(
                            ps,
                            xT[dc][hf][:, col : col + 128],
                            gw_s[:, it, dc, :],
                            start=(dc == 0),
                            stop=(dc == 1),
                        )
                    nc.scalar.activation(out=hS[ncx], in_=ps, func=AF.Copy)

            def agg_update(it, rt):
                pu = [
                    ps_u.tile([128, RT], f32, name=f"pu{dc}", tag="pu")
                    for dc in range(2)
                ]
                for ncx in range(NCH):
                    for dc in range(2):
                        nc.tensor.matmul(
                            pu[dc],
                            hS[ncx][:, dc * 128 : (dc + 1) * 128],
                            eT[ncx][:, rt * RT : (rt + 1) * RT],
                            start=(ncx == 0),
                            stop=(ncx == NCH - 1),
                        )
                for dc in range(2):
                    t = upd.tile([128, RT], f32, name="updt", tag="updt")
                    nc.vector.tensor_mul(
                        t, pu[dc], rinvB[:, rt * RT : (rt + 1) * RT]
                    )
                    nc.scalar.activation(
                        out=t,
                        in_=t,
                        func=AF.Relu,
                        bias=gb_s[:, it, dc, :],
                        scale=1.0,
                    )
                    nc.vector.tensor_add(
                        out=xT[dc][0][:, rt * RT : (rt + 1) * RT],
                        in0=xT[dc][0][:, rt * RT : (rt + 1) * RT],
                        in1=t,
                    )
                if it == 0:
                    for dc in range(2):
                        nc.sync.dma_start(
                            out=cc_in[
                                dc * 128 : (dc + 1) * 128, rt * RT : (rt + 1) * RT
                            ],
                            in_=xT[dc][0][:, rt * RT : (rt + 1) * RT],
                        )

            # pipeline: scores(rt0) -> h1 -> per-rt [R, agg1, update1] with
            # scores(rt+1) interleaved
            scores_exp(0)
            h_gen(0, 0)
            h_gen(0, 1)
            for rt in range(NRT):
                if rt + 1 < NRT:
                    scores_exp(rt + 1)
                r_reduce(rt)
                agg_update(0, rt)

            nc.gpsimd.collective_compute(
                "AllGather",
                mybir.AluOpType.bypass,
                replica_groups=[[0, 1], [2, 3], [4, 5], [6, 7]],
                ins=[cc_in[:, :].opt()],
                outs=[cc_out[:, :].opt()],
            )
            # h2 for my half can start while the collective flies
            h_gen(1, 0)
            # partner half = rank0_block * m1 + rank1_block * m0
            with tc.tile_pool(name="cct", bufs=4) as cct:
                for ct in range(NRT):
                    for dc in range(2):
                        t0 = cct.tile([128, RT], bf16, name="t0", tag="cct")
                        t1 = cct.tile([128, RT], bf16, name="t1", tag="cct")
                        nc.sync.dma_start(
                            out=t0,
                            in_=cc_out[
                                dc * 128 : (dc + 1) * 128,
                                ct * RT : (ct + 1) * RT,
                            ],
                        )
                        nc.sync.dma_start(
                            out=t1,
                            in_=cc_out[
                                256 + dc * 128 : 256 + (dc + 1) * 128,
                                ct * RT : (ct + 1) * RT,
                            ],
                        )
                        nc.vector.tensor_scalar_mul(t0, t0, m1_s)
                        nc.vector.tensor_scalar_mul(t1, t1, m0_s)
                        nc.vector.tensor_add(
                            out=xT[dc][1][:, ct * RT : (ct + 1) * RT],
                            in0=t0,
                            in1=t1,
                        )
            h_gen(1, 1)
            for rt in range(NRT):
                agg_update(1, rt)

            # ---- output partial: part = x2[mine] @ aggW_h ----
            with tc.tile_pool(name="ost", bufs=3) as ost:
                for rc in range(RH // 128):
                    ps = ps_h.tile([128, D], f32, name="pso", tag="psh")
                    for dc in range(2):
                        nc.tensor.matmul(
                            ps,
                            xT[dc][0][:, rc * 128 : (rc + 1) * 128],
                            agg_s[:, dc, :],
                            start=(dc == 0),
                            stop=(dc == 1),
                        )
                    ot = ost.tile([128, D], f32, name="ot", tag="ot")
                    nc.scalar.activation(out=ot, in_=ps, func=AF.Copy)
                    nc.sync.dma_start(
                        out=part[rc * 128 : (rc + 1) * 128, :], in_=ot
                    )

    _split_excess_waits(nc, mybir)
    return nc


def _get_nc():
    if "nc" not in _CACHE:
        _CACHE["nc"] = _build()
    return _CACHE["nc"]


def _in_maps(inputs):
    ne = np.asarray(inputs["nodes_embed"], dtype=np.float32)
    wq_w = np.asarray(inputs["WQ_w"], dtype=np.float32)
    wq_b = np.asarray(inputs["WQ_b"], dtype=np.float32)
    wk_w = np.asarray(inputs["WK_w"], dtype=np.float32)
    wk_b = np.asarray(inputs["WK_b"], dtype=np.float32)
    gcn_w = np.asarray(inputs["gcn_W"], dtype=np.float32)
    gcn_b = np.asarray(inputs["gcn_b"], dtype=np.float32)
    agg_w = np.asarray(inputs["agg_W"], dtype=np.float32)

    gb = np.ascontiguousarray(gcn_b.reshape(ITERS, 2, 128, 1))
    maps = []
    for c in range(8):
        b, h, rh = c // 4, (c // 2) % 2, c % 2
        if rh == 0:
            nodes = ne[b]
        else:
            nodes = np.concatenate([ne[b, RH:], ne[b, :RH]], axis=0)
        m0 = np.full((128, 1), 1.0 if rh == 0 else 0.0, np.float32)
        m1 = np.full((128, 1), 0.0 if rh == 0 else 1.0, np.float32)
        maps.append(
            {
                "nodes": np.ascontiguousarray(nodes),
                "wq": np.ascontiguousarray(wq_w[:, h * DK : (h + 1) * DK]),
                "wk": np.ascontiguousarray(wk_w[:, h * DK : (h + 1) * DK]),
                "wqb": np.ascontiguousarray(
                    wq_b[h * DK : (h + 1) * DK].reshape(DK, 1)
                ),
                "wkb": np.ascontiguousarray(
                    wk_b[h * DK : (h + 1) * DK].reshape(DK, 1)
                ),
                "gw": gcn_w,
                "gb": gb,
                "agg": np.ascontiguousarray(agg_w[h * D : (h + 1) * D, :]),
                "m0": m0,
                "m1": m1,
            }
        )
    return maps


def kernel(trace=False, tmpdir=None, **inputs):
    from concourse.bass_utils import run_bass_kernel_spmd

    nc = _get_nc()
    maps = _in_maps(inputs)
    kw = {}
    if trace:
        kw = dict(trace=True, tmpdir=tmpdir)
    res = run_bass_kernel_spmd(nc, maps, core_ids=list(range(8)), **kw)

    agg_b = np.asarray(inputs["agg_b"], dtype=np.float32)
    out = np.zeros((B, N, D), np.float32)
    for b in range(B):
        for rh in range(2):
            rows = slice(rh * RH, (rh + 1) * RH)
            out[b, rows, :] = (
                res.results[4 * b + 0 * 2 + rh]["part"]
                + res.results[4 * b + 1 * 2 + rh]["part"]
                + agg_b
            )
    if trace:
        return out, res
    return out
